# revision 18
# baseline (speedup 1.0000x reference)
"""Trainium2 Bass kernel for the DifferentiableModalPlate problem.

Reference computes, for 6400 plate modes j and T time samples t:
    disp[t] = sum_j A_j * exp(-sigma_j*K*(t-1)) * sin(omega_j*K*t)
    out     = disp / (max|disp| + 1e-8)

Device strategy (modes sharded over 8 cores, per sharding hint):
  Split t = C*c + d (chunks of C=128 samples). Angle addition gives
    wave_j(t) = a_j(c)*F_j(d) + b_j(c)*G_j(d)
  with per-mode chunk coefficients a,b and a per-mode time basis F,G:
    F_j(d) = exp(-sigma_j*K*d)*cos(omega_j*K*d)
    G_j(d) = exp(-sigma_j*K*d)*sin(omega_j*K*d)
    a_j(c) = A_j*exp(-sigma_j*K*(C*c-1))*sin(omega_j*K*C*c)
    b_j(c) = A_j*exp(-sigma_j*K*(C*c-1))*cos(omega_j*K*C*c)
  The O(modes*T) heavy sum over modes becomes PE matmuls:
    disp[d, c] = F^T a + G^T b   (contraction over modes, PSUM-accumulated)
  Each core owns a slab of modes; partial sums are AllReduce'd across the
  8 cores, then peak-normalized on device.

The tiny per-mode tables (O(modes*sqrt(T))) are precomputed on host in f64.
"""

import sys

sys.path.insert(0, "/opt/trn_rl_repo")

import numpy as np

import concourse.bass as bass
import concourse.bacc as bacc
import concourse.bass_isa as bass_isa
import concourse.mybir as mybir
import concourse.tile as tile
from concourse.bass_utils import run_bass_kernel_spmd

N_CORES = 8
C = 128  # samples per chunk == basis length == PE contraction M
F32 = mybir.dt.float32

# physics constants (from the nn.Module)
SR = 44100
K = 1.0 / SR
LX = 0.5
MAX_OM = 10000.0 * 2.0 * np.pi
MIN_OM = 20.0 * 2.0 * np.pi
OM2SQ = (2.0 * np.pi * 500.0) ** 2
ALPHA = 3.0 * np.log(10.0) / OM2SQ * (OM2SQ / 6.0)
BETA = 3.0 * np.log(10.0) / OM2SQ * (1.0 / 1.0 - 1.0 / 6.0)
MU_SCALE, DMU_SCALE, T0MU_SCALE = 2.43, 0.002452, 0.004115
M_MAX = 80

_NC_CACHE: dict = {}


def _softplus(x):
    return np.logaddexp(0.0, x)


def _sigmoid(x):
    return 1.0 / (1.0 + np.exp(-x))


def _mode_tables(mu_raw, D_raw, T0_raw, Ly_raw, xo_raw, yo_raw):
    """Per-mode omega, sigma, amplitude A (f64), invalid modes dropped."""
    mu = (_softplus(mu_raw) + 1e-4) * MU_SCALE
    D_over_mu = (_softplus(D_raw) + 1e-4) * DMU_SCALE
    T0_over_mu = (_softplus(T0_raw) + 1e-4) * T0MU_SCALE
    Ly = 1.1 + (4.0 - 1.1) * _sigmoid(Ly_raw)
    xo = 0.49 * LX + (1.0 - 0.49) * LX * _sigmoid(xo_raw)
    yo = 0.51 * Ly + (1.0 - 0.51) * Ly * _sigmoid(yo_raw)
    xi = 0.1 * LX
    yi = 0.1 * Ly
    idx = np.arange(1, M_MAX + 1, dtype=np.float64)
    gm, gn = np.meshgrid(idx, idx, indexing="ij")
    m, n = gm.ravel(), gn.ravel()
    g1 = (m * np.pi / LX) ** 2 + (n * np.pi / Ly) ** 2
    omega_sq = T0_over_mu * g1 + D_over_mu * g1 * g1
    omega = np.sqrt(np.maximum(omega_sq, 0.0))
    valid = (omega <= MAX_OM) & (omega >= MIN_OM)
    InW = np.cos(xi * np.pi * m / LX) * np.cos(yi * np.pi * n / Ly)
    OutW = np.cos(xo * np.pi * m / LX) * np.cos(yo * np.pi * n / Ly)
    sigma = ALPHA + BETA * omega**2
    ms = 0.25 * mu * LX * Ly
    P = OutW * InW * (K * K) * np.exp(-sigma * K) / ms
    A = P / (np.sin(omega * K) + 1e-8)
    return omega[valid], sigma[valid], A[valid]


def _build_nc(n_tiles: int, nch: int, pad_di: int):
    """SPMD program: per-core matmul partial sums + AllReduce + normalize.

    n_tiles: 128-mode tiles per core; nch: number of C-sample chunks;
    pad_di: first invalid d in the last chunk (128 if none).
    """
    key = (n_tiles, nch, pad_di)
    if key in _NC_CACHE:
        return _NC_CACHE[key]

    nc = bacc.Bacc("TRN2", target_bir_lowering=False, debug=False, num_devices=N_CORES)
    basis_d = nc.dram_tensor("basis", [128, 2 * n_tiles * C], F32, kind="ExternalInput")
    coef_d = nc.dram_tensor("coef", [128, 2 * n_tiles * nch], F32, kind="ExternalInput")
    disp_d = nc.dram_tensor("disp", [128, nch], F32, kind="ExternalOutput")
    first_add = rsem = lsem = None

    with tile.TileContext(nc, num_cores=N_CORES) as tc:
        with (
            tc.tile_pool(name="sbuf", bufs=1) as sp,
            tc.tile_pool(name="psum", bufs=1, space="PSUM") as pp,
            tc.tile_pool(name="dram", bufs=1, space="DRAM") as dp,
        ):
            # 1-byte entry AllGather: never read, exists so the NEFF carries a
            # real ncfw collective — that makes the runtime gang-launch all 8
            # cores (otherwise per-core dispatch skew reaches milliseconds,
            # which the remote-sem waits would absorb as dead time). Its
            # ~60us ncfw cost runs on the TOPSP/CC stream, off our engines.
            ag_src = sp.tile([1, 1], mybir.dt.uint8)
            nc.vector.memset(ag_src[:], 0)
            ag_in = dp.tile([1, 1], mybir.dt.uint8)
            ag_out = dp.tile([2, 1], mybir.dt.uint8)
            nc.gpsimd.dma_start(ag_in[:], ag_src[:])
            nc.gpsimd.collective_compute(
                "AllGather",
                mybir.AluOpType.bypass,
                replica_groups=[[2 * i, 2 * i + 1] for i in range(N_CORES // 2)],
                ins=[ag_in.opt()],
                outs=[ag_out.opt()],
            )

            bas = sp.tile([128, 2 * n_tiles * C], F32)
            nc.sync.dma_start(bas[:], basis_d[:])
            cof = sp.tile([128, 2 * n_tiles * nch], F32)
            nc.sync.dma_start(cof[:], coef_d[:])

            ps = pp.tile([128, nch], F32)
            nmm = 2 * n_tiles
            for i in range(nmm):
                nc.tensor.matmul(
                    ps[:],
                    lhsT=bas[:, i * C : (i + 1) * C],
                    rhs=cof[:, i * nch : (i + 1) * nch],
                    start=(i == 0),
                    stop=(i == nmm - 1),
                )

            part = sp.tile([128, nch], F32)
            nc.vector.tensor_copy(part[:], ps[:])

            import os as _os

            tot = sp.tile([128, nch], F32)
            if _os.environ.get("MODAL_USE_NCFW_CC"):
                # fallback: runtime (ncfw) AllReduce — correct but pays a
                # ~50us entry barrier + ~15us RDH on this runtime
                bounce_in = dp.tile([128, nch], F32)
                bounce_out = dp.tile([128, nch], F32)
                nc.gpsimd.dma_start(bounce_in[:], part[:])
                nc.gpsimd.collective_compute(
                    "AllReduce",
                    mybir.AluOpType.add,
                    replica_groups=[list(range(N_CORES))],
                    ins=[bounce_in.opt()],
                    outs=[bounce_out.opt()],
                )
                nc.sync.dma_start(tot[:], bounce_out[:])
            else:
                # all-to-all partial exchange over SBUF-to-SBUF remote DMA,
                # bypassing the ncfw collective path entirely. Every core
                # broadcasts its partial to the 7 peers (XOR-relative dests;
                # transfer k rides a distinct SDMA engine pair, so all 7 run
                # concurrently) and sums the 7 received partials locally.
                # No entry barrier is needed: a peer's recv buffers and
                # semaphores are only touched by writes that are themselves
                # gated behind ~20us of sender-side work, kernel executions
                # are host-serialized, and nothing clears semaphores at
                # kernel entry in this (non-target_bir_lowering) config.
                rsem = nc.alloc_semaphore("modal_rsem")
                lsem = nc.alloc_semaphore("modal_lsem")
                recv = {}
                for k in range(1, N_CORES):
                    recv[k] = sp.tile(
                        [128, nch], F32, name=f"recv{k}", tag=f"recv{k}"
                    )
                for k in range(1, N_CORES):
                    rdests: list = [None] * N_CORES
                    rdests[k] = (0, k)
                    nc.gpsimd.remote_dma_broadcast(
                        recv[k][:], part[:], rsem, lsem, rdests=rdests
                    )
                nc.gpsimd.trigger_dma(count=None)
                first_add = nc.vector.tensor_add(tot[:], part[:], recv[1][:])
                for k in range(2, N_CORES):
                    nc.vector.tensor_add(tot[:], tot[:], recv[k][:])

            # peak over the valid t < num_samples region only: the last
            # chunk's padded tail (d >= pad_di) must not feed the max
            pk = sp.tile([128, 1], F32)
            if pad_di < 128:
                nc.vector.tensor_reduce(
                    pk[:], tot[:, 0 : nch - 1], axis=mybir.AxisListType.X,
                    op=mybir.AluOpType.max, apply_absolute_value=True,
                )
                pkl = sp.tile([128, 1], F32)
                nc.vector.tensor_reduce(
                    pkl[0:pad_di, :], tot[0:pad_di, nch - 1 : nch],
                    axis=mybir.AxisListType.X,
                    op=mybir.AluOpType.max, apply_absolute_value=True,
                )
                nc.vector.tensor_max(
                    pk[0:pad_di, :], pk[0:pad_di, :], pkl[0:pad_di, :]
                )
            else:
                nc.vector.tensor_reduce(
                    pk[:], tot[:], axis=mybir.AxisListType.X,
                    op=mybir.AluOpType.max, apply_absolute_value=True,
                )
            pkg = sp.tile([128, 1], F32)
            nc.gpsimd.partition_all_reduce(
                pkg[:], pk[:], channels=128, reduce_op=bass_isa.ReduceOp.absmax
            )
            pke = sp.tile([128, 1], F32)
            nc.vector.tensor_scalar_add(pke[:], pkg[:], 1e-8)
            inv = sp.tile([128, 1], F32)
            nc.vector.reciprocal(inv[:], pke[:])

            outt = sp.tile([128, nch], F32)
            nc.vector.tensor_scalar_mul(outt[:], tot[:], inv[:])
            nc.sync.dma_start(disp_d[:], outt[:])

    if first_add is not None:
        # Patch in the remote-arrival gate AFTER Tile scheduling: the Tile
        # single-core sim cannot model cross-core sem increments and would
        # report a deadlock. Each of the 7 peers incs rsem by 16//8 = 2.
        # The add's wait slots are already used by Tile, so emit a standalone
        # vector-engine wait and splice it in right before the first add.
        nsem = 2 * (N_CORES - 1)
        gate = nc.vector.wait_ge(rsem, nsem)
        target = first_add.ins.name
        moved = False
        for bb in nc.main_func.blocks:
            names = [i.name for i in bb.instructions]
            if target in names and gate.ins.name in names:
                bb.instructions.remove(gate.ins)
                bb.instructions.insert(
                    bb.instructions.index(first_add.ins), gate.ins
                )
                moved = True
                break
            if target in names:
                # gate landed in a different (later) block; move it here
                for bb2 in nc.main_func.blocks:
                    if gate.ins in bb2.instructions:
                        bb2.instructions.remove(gate.ins)
                bb.instructions.insert(
                    bb.instructions.index(first_add.ins), gate.ins
                )
                moved = True
                break
        assert moved, "failed to splice remote-arrival gate"
        # Cleanup appended after Tile's final all-engine barrier (so it runs
        # strictly after the adds): leave both sems at 0 for any subsequent
        # execution of this NEFF.
        nc.gpsimd.sem_clear(rsem)._wait_ge(rsem, nsem)
        nc.gpsimd.sem_clear(lsem)._wait_ge(lsem, 16 * (N_CORES - 1))
        # coordinated multi-core launch: without this flag the runtime
        # dispatches the 8 per-core executions with multi-ms skew, which the
        # remote-sem waits then absorb as dead time
        nc.has_collectives = True

    nc.compile()
    _NC_CACHE[key] = nc
    return nc


def _tile_pack(slab: np.ndarray, n_tiles: int) -> np.ndarray:
    """[n_tiles*128, W] -> [128, n_tiles*W] so tile i sits at cols [i*W,(i+1)*W)."""
    w = slab.shape[1]
    return (
        slab.reshape(n_tiles, 128, w).transpose(1, 0, 2).reshape(128, n_tiles * w)
    )


def _install_ntff_hook_shim():
    """The RL container's antenv lacks axon_hooks, so bass_utils' trace=True
    path can't find the NTFF profile hook. Recreate it from trn_agent_boot's
    ctypes shim against the injected libaxon_pjrt.so."""
    import sys as _sys
    import types

    if "antenv.axon_hooks" in _sys.modules:
        return
    try:
        from trn_agent_boot.trn_boot import _ntff_profile_via_ctypes

        hook = _ntff_profile_via_ctypes("/opt/axon/libaxon_pjrt.so")
    except Exception:
        hook = None
    mod = types.ModuleType("antenv.axon_hooks")
    mod._hook = hook
    mod.get_axon_ntff_profile_hook = lambda: mod._hook
    mod.set_axon_ntff_profile_hook = lambda h: setattr(mod, "_hook", h)
    _sys.modules["antenv.axon_hooks"] = mod


def kernel(
    mu_raw, D_over_mu_raw, T0_over_mu_raw, Ly_raw, xo_raw, yo_raw, num_samples
) -> np.ndarray:
    mu_raw = float(np.asarray(mu_raw))
    D_raw = float(np.asarray(D_over_mu_raw))
    T0_raw = float(np.asarray(T0_over_mu_raw))
    Ly_raw = float(np.asarray(Ly_raw))
    xo_raw = float(np.asarray(xo_raw))
    yo_raw = float(np.asarray(yo_raw))
    T = int(np.asarray(num_samples))

    omega, sigma, A = _mode_tables(mu_raw, D_raw, T0_raw, Ly_raw, xo_raw, yo_raw)
    n_valid = omega.shape[0]
    per_core = ((n_valid + N_CORES * 128 - 1) // (N_CORES * 128)) * 128
    n_tiles = per_core // 128
    n_pad = per_core * N_CORES
    omega = np.pad(omega, (0, n_pad - n_valid))
    sigma = np.pad(sigma, (0, n_pad - n_valid))
    A = np.pad(A, (0, n_pad - n_valid))

    nch = (T + C - 1) // C
    pad_di = T - C * (nch - 1)  # valid d's in last chunk; 128 if exact fit

    # host tables in f64, cast to f32
    d = np.arange(C, dtype=np.float64)
    ph = omega[:, None] * K * d[None, :]
    env = np.exp(-sigma[:, None] * K * d[None, :])
    F = (env * np.cos(ph)).astype(np.float32)  # [n_pad, C]
    G = (env * np.sin(ph)).astype(np.float32)

    t0 = np.arange(nch, dtype=np.float64) * C
    th = omega[:, None] * K * t0[None, :]
    cenv = A[:, None] * np.exp(-sigma[:, None] * K * (t0[None, :] - 1.0))
    a = (cenv * np.sin(th)).astype(np.float32)  # [n_pad, nch]
    b = (cenv * np.cos(th)).astype(np.float32)

    nc = _build_nc(n_tiles, nch, pad_di)

    in_maps = []
    for r in range(N_CORES):
        sl = slice(r * per_core, (r + 1) * per_core)
        basis = np.concatenate(
            [_tile_pack(F[sl], n_tiles), _tile_pack(G[sl], n_tiles)], axis=1
        )
        coef = np.concatenate(
            [_tile_pack(a[sl], n_tiles), _tile_pack(b[sl], n_tiles)], axis=1
        )
        in_maps.append(
            {
                "basis": np.ascontiguousarray(basis),
                "coef": np.ascontiguousarray(coef),
            }
        )

    import os

    trace = bool(os.environ.get("MODAL_KERNEL_TRACE"))
    if trace:
        _install_ntff_hook_shim()
    res = run_bass_kernel_spmd(
        nc, in_maps, core_ids=list(range(N_CORES)), trace=trace
    )
    kernel._last_results = res  # for profiling from test.py
    out = res.results[0]["disp"]  # [128, nch], element (d, c) = disp[C*c+d]
    return np.ascontiguousarray(out.T.reshape(-1)[:T]).astype(np.float32)


if __name__ == "__main__":
    z = np.zeros((), np.float32)
    y = kernel(z, z, z, z, z, z, 22050)
    print(y.shape, y.dtype, y[:5], np.max(np.abs(y)))


# revision 20
# speedup vs baseline: 197.0826x; 197.0826x over previous
"""Trainium2 Bass kernel for the DifferentiableModalPlate problem.

Reference computes, for 6400 plate modes j and T time samples t:
    disp[t] = sum_j A_j * exp(-sigma_j*K*(t-1)) * sin(omega_j*K*t)
    out     = disp / (max|disp| + 1e-8)

Device strategy (modes sharded over 8 cores, per sharding hint):
  Split t = C*c + d (chunks of C=128 samples). Angle addition gives
    wave_j(t) = a_j(c)*F_j(d) + b_j(c)*G_j(d)
  with per-mode chunk coefficients a,b and a per-mode time basis F,G:
    F_j(d) = exp(-sigma_j*K*d)*cos(omega_j*K*d)
    G_j(d) = exp(-sigma_j*K*d)*sin(omega_j*K*d)
    a_j(c) = A_j*exp(-sigma_j*K*(C*c-1))*sin(omega_j*K*C*c)
    b_j(c) = A_j*exp(-sigma_j*K*(C*c-1))*cos(omega_j*K*C*c)
  The O(modes*T) heavy sum over modes becomes PE matmuls:
    disp[d, c] = F^T a + G^T b   (contraction over modes, PSUM-accumulated)
  Each core owns a slab of modes; partial sums are AllReduce'd across the
  8 cores, then peak-normalized on device.

The tiny per-mode tables (O(modes*sqrt(T))) are precomputed on host in f64.
"""

import sys

sys.path.insert(0, "/opt/trn_rl_repo")

import numpy as np

import concourse.bass as bass
import concourse.bacc as bacc
import concourse.bass_isa as bass_isa
import concourse.mybir as mybir
import concourse.tile as tile
from concourse.bass_utils import run_bass_kernel_spmd

N_CORES = 8
C = 128  # samples per chunk == basis length == PE contraction M
F32 = mybir.dt.float32

# physics constants (from the nn.Module)
SR = 44100
K = 1.0 / SR
LX = 0.5
MAX_OM = 10000.0 * 2.0 * np.pi
MIN_OM = 20.0 * 2.0 * np.pi
OM2SQ = (2.0 * np.pi * 500.0) ** 2
ALPHA = 3.0 * np.log(10.0) / OM2SQ * (OM2SQ / 6.0)
BETA = 3.0 * np.log(10.0) / OM2SQ * (1.0 / 1.0 - 1.0 / 6.0)
MU_SCALE, DMU_SCALE, T0MU_SCALE = 2.43, 0.002452, 0.004115
M_MAX = 80

_NC_CACHE: dict = {}


def _softplus(x):
    return np.logaddexp(0.0, x)


def _sigmoid(x):
    return 1.0 / (1.0 + np.exp(-x))


def _mode_tables(mu_raw, D_raw, T0_raw, Ly_raw, xo_raw, yo_raw):
    """Per-mode omega, sigma, amplitude A (f64), invalid modes dropped."""
    mu = (_softplus(mu_raw) + 1e-4) * MU_SCALE
    D_over_mu = (_softplus(D_raw) + 1e-4) * DMU_SCALE
    T0_over_mu = (_softplus(T0_raw) + 1e-4) * T0MU_SCALE
    Ly = 1.1 + (4.0 - 1.1) * _sigmoid(Ly_raw)
    xo = 0.49 * LX + (1.0 - 0.49) * LX * _sigmoid(xo_raw)
    yo = 0.51 * Ly + (1.0 - 0.51) * Ly * _sigmoid(yo_raw)
    xi = 0.1 * LX
    yi = 0.1 * Ly
    idx = np.arange(1, M_MAX + 1, dtype=np.float64)
    gm, gn = np.meshgrid(idx, idx, indexing="ij")
    m, n = gm.ravel(), gn.ravel()
    g1 = (m * np.pi / LX) ** 2 + (n * np.pi / Ly) ** 2
    omega_sq = T0_over_mu * g1 + D_over_mu * g1 * g1
    omega = np.sqrt(np.maximum(omega_sq, 0.0))
    valid = (omega <= MAX_OM) & (omega >= MIN_OM)
    InW = np.cos(xi * np.pi * m / LX) * np.cos(yi * np.pi * n / Ly)
    OutW = np.cos(xo * np.pi * m / LX) * np.cos(yo * np.pi * n / Ly)
    sigma = ALPHA + BETA * omega**2
    ms = 0.25 * mu * LX * Ly
    P = OutW * InW * (K * K) * np.exp(-sigma * K) / ms
    A = P / (np.sin(omega * K) + 1e-8)
    return omega[valid], sigma[valid], A[valid]


def _build_nc(n_tiles: int, nch: int, pad_di: int):
    """SPMD program: per-core matmul partial sums + AllReduce + normalize.

    n_tiles: 128-mode tiles per core; nch: number of C-sample chunks;
    pad_di: first invalid d in the last chunk (128 if none).
    """
    key = (n_tiles, nch, pad_di)
    if key in _NC_CACHE:
        return _NC_CACHE[key]

    nc = bacc.Bacc("TRN2", target_bir_lowering=False, debug=False, num_devices=N_CORES)
    basis_d = nc.dram_tensor("basis", [128, 2 * n_tiles * C], F32, kind="ExternalInput")
    coef_d = nc.dram_tensor("coef", [128, 2 * n_tiles * nch], F32, kind="ExternalInput")
    disp_d = nc.dram_tensor("disp", [128, nch], F32, kind="ExternalOutput")
    first_add = rsem = lsem = None

    with tile.TileContext(nc, num_cores=N_CORES) as tc:
        with (
            tc.tile_pool(name="sbuf", bufs=1) as sp,
            tc.tile_pool(name="psum", bufs=1, space="PSUM") as pp,
            tc.tile_pool(name="dram", bufs=1, space="DRAM") as dp,
        ):
            bas = sp.tile([128, 2 * n_tiles * C], F32)
            nc.sync.dma_start(bas[:], basis_d[:])
            cof = sp.tile([128, 2 * n_tiles * nch], F32)
            nc.sync.dma_start(cof[:], coef_d[:])

            ps = pp.tile([128, nch], F32)
            nmm = 2 * n_tiles
            for i in range(nmm):
                nc.tensor.matmul(
                    ps[:],
                    lhsT=bas[:, i * C : (i + 1) * C],
                    rhs=cof[:, i * nch : (i + 1) * nch],
                    start=(i == 0),
                    stop=(i == nmm - 1),
                )

            part = sp.tile([128, nch], F32)
            nc.vector.tensor_copy(part[:], ps[:])

            import os as _os

            tot = sp.tile([128, nch], F32)
            if not _os.environ.get("MODAL_USE_RDMA_CC"):
                # runtime (ncfw) AllReduce. It pays a ~40-70us entry barrier
                # plus ~15us of RDH data movement, but the barrier is also
                # what gang-launches the 8 cores: without any ncfw collective
                # in the NEFF the per-core dispatch skew on this runtime
                # reaches milliseconds, which any hand-rolled cross-core
                # exchange then absorbs as dead time. Measured end to end this
                # is the fastest reliable option.
                bounce_in = dp.tile([128, nch], F32)
                bounce_out = dp.tile([128, nch], F32)
                nc.gpsimd.dma_start(bounce_in[:], part[:])
                nc.gpsimd.collective_compute(
                    "AllReduce",
                    mybir.AluOpType.add,
                    replica_groups=[list(range(N_CORES))],
                    ins=[bounce_in.opt()],
                    outs=[bounce_out.opt()],
                )
                nc.sync.dma_start(tot[:], bounce_out[:])
            else:
                # all-to-all partial exchange over SBUF-to-SBUF remote DMA,
                # bypassing the ncfw collective path entirely. Every core
                # broadcasts its partial to the 7 peers (XOR-relative dests;
                # transfer k rides a distinct SDMA engine pair, so all 7 run
                # concurrently) and sums the 7 received partials locally.
                # No entry barrier is needed: a peer's recv buffers and
                # semaphores are only touched by writes that are themselves
                # gated behind ~20us of sender-side work, kernel executions
                # are host-serialized, and nothing clears semaphores at
                # kernel entry in this (non-target_bir_lowering) config.
                rsem = nc.alloc_semaphore("modal_rsem")
                lsem = nc.alloc_semaphore("modal_lsem")
                recv = {}
                for k in range(1, N_CORES):
                    recv[k] = sp.tile(
                        [128, nch], F32, name=f"recv{k}", tag=f"recv{k}"
                    )
                for k in range(1, N_CORES):
                    rdests: list = [None] * N_CORES
                    rdests[k] = (0, k)
                    nc.gpsimd.remote_dma_broadcast(
                        recv[k][:], part[:], rsem, lsem, rdests=rdests
                    )
                nc.gpsimd.trigger_dma(count=None)
                first_add = nc.vector.tensor_add(tot[:], part[:], recv[1][:])
                for k in range(2, N_CORES):
                    nc.vector.tensor_add(tot[:], tot[:], recv[k][:])

            # peak over the valid t < num_samples region only: the last
            # chunk's padded tail (d >= pad_di) must not feed the max
            pk = sp.tile([128, 1], F32)
            if pad_di < 128:
                nc.vector.tensor_reduce(
                    pk[:], tot[:, 0 : nch - 1], axis=mybir.AxisListType.X,
                    op=mybir.AluOpType.max, apply_absolute_value=True,
                )
                pkl = sp.tile([128, 1], F32)
                nc.vector.tensor_reduce(
                    pkl[0:pad_di, :], tot[0:pad_di, nch - 1 : nch],
                    axis=mybir.AxisListType.X,
                    op=mybir.AluOpType.max, apply_absolute_value=True,
                )
                nc.vector.tensor_max(
                    pk[0:pad_di, :], pk[0:pad_di, :], pkl[0:pad_di, :]
                )
            else:
                nc.vector.tensor_reduce(
                    pk[:], tot[:], axis=mybir.AxisListType.X,
                    op=mybir.AluOpType.max, apply_absolute_value=True,
                )
            pkg = sp.tile([128, 1], F32)
            nc.gpsimd.partition_all_reduce(
                pkg[:], pk[:], channels=128, reduce_op=bass_isa.ReduceOp.absmax
            )
            pke = sp.tile([128, 1], F32)
            nc.vector.tensor_scalar_add(pke[:], pkg[:], 1e-8)
            inv = sp.tile([128, 1], F32)
            nc.vector.reciprocal(inv[:], pke[:])

            outt = sp.tile([128, nch], F32)
            nc.vector.tensor_scalar_mul(outt[:], tot[:], inv[:])
            nc.sync.dma_start(disp_d[:], outt[:])

    if first_add is not None:
        # Patch in the remote-arrival gate AFTER Tile scheduling: the Tile
        # single-core sim cannot model cross-core sem increments and would
        # report a deadlock. Each of the 7 peers incs rsem by 16//8 = 2.
        # The add's wait slots are already used by Tile, so emit a standalone
        # vector-engine wait and splice it in right before the first add.
        nsem = 2 * (N_CORES - 1)
        gate = nc.vector.wait_ge(rsem, nsem)
        target = first_add.ins.name
        moved = False
        for bb in nc.main_func.blocks:
            names = [i.name for i in bb.instructions]
            if target in names and gate.ins.name in names:
                bb.instructions.remove(gate.ins)
                bb.instructions.insert(
                    bb.instructions.index(first_add.ins), gate.ins
                )
                moved = True
                break
            if target in names:
                # gate landed in a different (later) block; move it here
                for bb2 in nc.main_func.blocks:
                    if gate.ins in bb2.instructions:
                        bb2.instructions.remove(gate.ins)
                bb.instructions.insert(
                    bb.instructions.index(first_add.ins), gate.ins
                )
                moved = True
                break
        assert moved, "failed to splice remote-arrival gate"
        # Cleanup appended after Tile's final all-engine barrier (so it runs
        # strictly after the adds): leave both sems at 0 for any subsequent
        # execution of this NEFF.
        nc.gpsimd.sem_clear(rsem)._wait_ge(rsem, nsem)
        nc.gpsimd.sem_clear(lsem)._wait_ge(lsem, 16 * (N_CORES - 1))
        # coordinated multi-core launch: without this flag the runtime
        # dispatches the 8 per-core executions with multi-ms skew, which the
        # remote-sem waits then absorb as dead time
        nc.has_collectives = True

    nc.compile()
    _NC_CACHE[key] = nc
    return nc


def _tile_pack(slab: np.ndarray, n_tiles: int) -> np.ndarray:
    """[n_tiles*128, W] -> [128, n_tiles*W] so tile i sits at cols [i*W,(i+1)*W)."""
    w = slab.shape[1]
    return (
        slab.reshape(n_tiles, 128, w).transpose(1, 0, 2).reshape(128, n_tiles * w)
    )


def _install_ntff_hook_shim():
    """The RL container's antenv lacks axon_hooks, so bass_utils' trace=True
    path can't find the NTFF profile hook. Recreate it from trn_agent_boot's
    ctypes shim against the injected libaxon_pjrt.so."""
    import sys as _sys
    import types

    if "antenv.axon_hooks" in _sys.modules:
        return
    try:
        from trn_agent_boot.trn_boot import _ntff_profile_via_ctypes

        hook = _ntff_profile_via_ctypes("/opt/axon/libaxon_pjrt.so")
    except Exception:
        hook = None
    mod = types.ModuleType("antenv.axon_hooks")
    mod._hook = hook
    mod.get_axon_ntff_profile_hook = lambda: mod._hook
    mod.set_axon_ntff_profile_hook = lambda h: setattr(mod, "_hook", h)
    _sys.modules["antenv.axon_hooks"] = mod


def kernel(
    mu_raw, D_over_mu_raw, T0_over_mu_raw, Ly_raw, xo_raw, yo_raw, num_samples
) -> np.ndarray:
    mu_raw = float(np.asarray(mu_raw))
    D_raw = float(np.asarray(D_over_mu_raw))
    T0_raw = float(np.asarray(T0_over_mu_raw))
    Ly_raw = float(np.asarray(Ly_raw))
    xo_raw = float(np.asarray(xo_raw))
    yo_raw = float(np.asarray(yo_raw))
    T = int(np.asarray(num_samples))

    omega, sigma, A = _mode_tables(mu_raw, D_raw, T0_raw, Ly_raw, xo_raw, yo_raw)
    n_valid = omega.shape[0]
    per_core = ((n_valid + N_CORES * 128 - 1) // (N_CORES * 128)) * 128
    n_tiles = per_core // 128
    n_pad = per_core * N_CORES
    omega = np.pad(omega, (0, n_pad - n_valid))
    sigma = np.pad(sigma, (0, n_pad - n_valid))
    A = np.pad(A, (0, n_pad - n_valid))

    nch = (T + C - 1) // C
    pad_di = T - C * (nch - 1)  # valid d's in last chunk; 128 if exact fit

    # host tables in f64, cast to f32
    d = np.arange(C, dtype=np.float64)
    ph = omega[:, None] * K * d[None, :]
    env = np.exp(-sigma[:, None] * K * d[None, :])
    F = (env * np.cos(ph)).astype(np.float32)  # [n_pad, C]
    G = (env * np.sin(ph)).astype(np.float32)

    t0 = np.arange(nch, dtype=np.float64) * C
    th = omega[:, None] * K * t0[None, :]
    cenv = A[:, None] * np.exp(-sigma[:, None] * K * (t0[None, :] - 1.0))
    a = (cenv * np.sin(th)).astype(np.float32)  # [n_pad, nch]
    b = (cenv * np.cos(th)).astype(np.float32)

    nc = _build_nc(n_tiles, nch, pad_di)

    in_maps = []
    for r in range(N_CORES):
        sl = slice(r * per_core, (r + 1) * per_core)
        basis = np.concatenate(
            [_tile_pack(F[sl], n_tiles), _tile_pack(G[sl], n_tiles)], axis=1
        )
        coef = np.concatenate(
            [_tile_pack(a[sl], n_tiles), _tile_pack(b[sl], n_tiles)], axis=1
        )
        in_maps.append(
            {
                "basis": np.ascontiguousarray(basis),
                "coef": np.ascontiguousarray(coef),
            }
        )

    import os

    trace = bool(os.environ.get("MODAL_KERNEL_TRACE"))
    if trace:
        _install_ntff_hook_shim()
    res = run_bass_kernel_spmd(
        nc, in_maps, core_ids=list(range(N_CORES)), trace=trace
    )
    kernel._last_results = res  # for profiling from test.py
    out = res.results[0]["disp"]  # [128, nch], element (d, c) = disp[C*c+d]
    return np.ascontiguousarray(out.T.reshape(-1)[:T]).astype(np.float32)


if __name__ == "__main__":
    z = np.zeros((), np.float32)
    y = kernel(z, z, z, z, z, z, 22050)
    print(y.shape, y.dtype, y[:5], np.max(np.abs(y)))


# revision 23
# speedup vs baseline: 213.2208x; 1.0819x over previous
"""Trainium2 Bass kernel for the DifferentiableModalPlate problem.

Reference computes, for 6400 plate modes j and T time samples t:
    disp[t] = sum_j A_j * exp(-sigma_j*K*(t-1)) * sin(omega_j*K*t)
    out     = disp / (max|disp| + 1e-8)

Device strategy (modes sharded over 8 cores, per sharding hint):
  Split t = C*c + d (chunks of C=128 samples). Angle addition gives
    wave_j(t) = a_j(c)*F_j(d) + b_j(c)*G_j(d)
  with per-mode chunk coefficients a,b and a per-mode time basis F,G:
    F_j(d) = exp(-sigma_j*K*d)*cos(omega_j*K*d)
    G_j(d) = exp(-sigma_j*K*d)*sin(omega_j*K*d)
    a_j(c) = A_j*exp(-sigma_j*K*(C*c-1))*sin(omega_j*K*C*c)
    b_j(c) = A_j*exp(-sigma_j*K*(C*c-1))*cos(omega_j*K*C*c)
  The O(modes*T) heavy sum over modes becomes PE matmuls:
    disp[d, c] = F^T a + G^T b   (contraction over modes, PSUM-accumulated)
  Each core owns a slab of modes; partial sums are AllReduce'd across the
  8 cores, then peak-normalized on device.

The tiny per-mode tables (O(modes*sqrt(T))) are precomputed on host in f64.
"""

import sys

sys.path.insert(0, "/opt/trn_rl_repo")

import numpy as np

import concourse.bass as bass
import concourse.bacc as bacc
import concourse.bass_isa as bass_isa
import concourse.mybir as mybir
import concourse.tile as tile
from concourse.bass_utils import run_bass_kernel_spmd

N_CORES = 8
C = 128  # samples per chunk == basis length == PE contraction M
F32 = mybir.dt.float32

# physics constants (from the nn.Module)
SR = 44100
K = 1.0 / SR
LX = 0.5
MAX_OM = 10000.0 * 2.0 * np.pi
MIN_OM = 20.0 * 2.0 * np.pi
OM2SQ = (2.0 * np.pi * 500.0) ** 2
ALPHA = 3.0 * np.log(10.0) / OM2SQ * (OM2SQ / 6.0)
BETA = 3.0 * np.log(10.0) / OM2SQ * (1.0 / 1.0 - 1.0 / 6.0)
MU_SCALE, DMU_SCALE, T0MU_SCALE = 2.43, 0.002452, 0.004115
M_MAX = 80

_NC_CACHE: dict = {}


def _softplus(x):
    return np.logaddexp(0.0, x)


def _sigmoid(x):
    return 1.0 / (1.0 + np.exp(-x))


def _mode_tables(mu_raw, D_raw, T0_raw, Ly_raw, xo_raw, yo_raw):
    """Per-mode omega, sigma, amplitude A (f64), invalid modes dropped."""
    mu = (_softplus(mu_raw) + 1e-4) * MU_SCALE
    D_over_mu = (_softplus(D_raw) + 1e-4) * DMU_SCALE
    T0_over_mu = (_softplus(T0_raw) + 1e-4) * T0MU_SCALE
    Ly = 1.1 + (4.0 - 1.1) * _sigmoid(Ly_raw)
    xo = 0.49 * LX + (1.0 - 0.49) * LX * _sigmoid(xo_raw)
    yo = 0.51 * Ly + (1.0 - 0.51) * Ly * _sigmoid(yo_raw)
    xi = 0.1 * LX
    yi = 0.1 * Ly
    idx = np.arange(1, M_MAX + 1, dtype=np.float64)
    gm, gn = np.meshgrid(idx, idx, indexing="ij")
    m, n = gm.ravel(), gn.ravel()
    g1 = (m * np.pi / LX) ** 2 + (n * np.pi / Ly) ** 2
    omega_sq = T0_over_mu * g1 + D_over_mu * g1 * g1
    omega = np.sqrt(np.maximum(omega_sq, 0.0))
    valid = (omega <= MAX_OM) & (omega >= MIN_OM)
    InW = np.cos(xi * np.pi * m / LX) * np.cos(yi * np.pi * n / Ly)
    OutW = np.cos(xo * np.pi * m / LX) * np.cos(yo * np.pi * n / Ly)
    sigma = ALPHA + BETA * omega**2
    ms = 0.25 * mu * LX * Ly
    P = OutW * InW * (K * K) * np.exp(-sigma * K) / ms
    A = P / (np.sin(omega * K) + 1e-8)
    return omega[valid], sigma[valid], A[valid]


def _build_nc(n_tiles: int, nch: int, pad_di: int):
    """SPMD program: per-core matmul partial sums + AllReduce + normalize.

    n_tiles: 128-mode tiles per core; nch: number of C-sample chunks;
    pad_di: first invalid d in the last chunk (128 if none).
    """
    key = (n_tiles, nch, pad_di)
    if key in _NC_CACHE:
        return _NC_CACHE[key]

    nc = bacc.Bacc("TRN2", target_bir_lowering=False, debug=False, num_devices=N_CORES)
    basis_d = nc.dram_tensor("basis", [128, 2 * n_tiles * C], F32, kind="ExternalInput")
    coef_d = nc.dram_tensor("coef", [128, 2 * n_tiles * nch], F32, kind="ExternalInput")
    disp_d = nc.dram_tensor("disp", [128, nch], F32, kind="ExternalOutput")

    with tile.TileContext(nc, num_cores=N_CORES) as tc:
        with (
            tc.tile_pool(name="sbuf", bufs=1) as sp,
            tc.tile_pool(name="psum", bufs=1, space="PSUM") as pp,
            tc.tile_pool(name="dram", bufs=1, space="DRAM") as dp,
        ):
            bas = sp.tile([128, 2 * n_tiles * C], F32)
            nc.sync.dma_start(bas[:], basis_d[:])
            cof = sp.tile([128, 2 * n_tiles * nch], F32)
            nc.sync.dma_start(cof[:], coef_d[:])

            ps = pp.tile([128, nch], F32)
            nmm = 2 * n_tiles
            for i in range(nmm):
                nc.tensor.matmul(
                    ps[:],
                    lhsT=bas[:, i * C : (i + 1) * C],
                    rhs=cof[:, i * nch : (i + 1) * nch],
                    start=(i == 0),
                    stop=(i == nmm - 1),
                )

            part = sp.tile([128, nch], F32)
            nc.vector.tensor_copy(part[:], ps[:])

            # Runtime (ncfw) AllReduce of the per-core partial sums. It pays
            # a ~40-70us entry barrier plus ~15us of RDH data movement, but
            # the collective is also what gang-launches the 8 cores: without
            # any ncfw collective in the NEFF, the per-core dispatch skew on
            # this runtime reaches milliseconds, which any hand-rolled
            # cross-core exchange (e.g. a remote-DMA all-to-all; tried) then
            # absorbs as dead time. The compute phase above fully hides under
            # the entry barrier, so this is the measured end-to-end optimum.
            tot = sp.tile([128, nch], F32)
            bounce_in = dp.tile([128, nch], F32)
            bounce_out = dp.tile([128, nch], F32)
            nc.gpsimd.dma_start(bounce_in[:], part[:])
            nc.gpsimd.collective_compute(
                "AllReduce",
                mybir.AluOpType.add,
                replica_groups=[list(range(N_CORES))],
                ins=[bounce_in.opt()],
                outs=[bounce_out.opt()],
            )
            nc.sync.dma_start(tot[:], bounce_out[:])

            # peak over the valid t < num_samples region only: the last
            # chunk's padded tail (d >= pad_di) must not feed the max
            pk = sp.tile([128, 1], F32)
            if pad_di < 128:
                nc.vector.tensor_reduce(
                    pk[:], tot[:, 0 : nch - 1], axis=mybir.AxisListType.X,
                    op=mybir.AluOpType.max, apply_absolute_value=True,
                )
                pkl = sp.tile([128, 1], F32)
                nc.vector.tensor_reduce(
                    pkl[0:pad_di, :], tot[0:pad_di, nch - 1 : nch],
                    axis=mybir.AxisListType.X,
                    op=mybir.AluOpType.max, apply_absolute_value=True,
                )
                nc.vector.tensor_max(
                    pk[0:pad_di, :], pk[0:pad_di, :], pkl[0:pad_di, :]
                )
            else:
                nc.vector.tensor_reduce(
                    pk[:], tot[:], axis=mybir.AxisListType.X,
                    op=mybir.AluOpType.max, apply_absolute_value=True,
                )
            pkg = sp.tile([128, 1], F32)
            nc.gpsimd.partition_all_reduce(
                pkg[:], pk[:], channels=128, reduce_op=bass_isa.ReduceOp.absmax
            )
            pke = sp.tile([128, 1], F32)
            nc.vector.tensor_scalar_add(pke[:], pkg[:], 1e-8)
            inv = sp.tile([128, 1], F32)
            nc.vector.reciprocal(inv[:], pke[:])

            outt = sp.tile([128, nch], F32)
            nc.vector.tensor_scalar_mul(outt[:], tot[:], inv[:])
            nc.sync.dma_start(disp_d[:], outt[:])

    nc.compile()
    _NC_CACHE[key] = nc
    return nc


def _tile_pack(slab: np.ndarray, n_tiles: int) -> np.ndarray:
    """[n_tiles*128, W] -> [128, n_tiles*W] so tile i sits at cols [i*W,(i+1)*W)."""
    w = slab.shape[1]
    return (
        slab.reshape(n_tiles, 128, w).transpose(1, 0, 2).reshape(128, n_tiles * w)
    )


def _install_ntff_hook_shim():
    """The RL container's antenv lacks axon_hooks, so bass_utils' trace=True
    path can't find the NTFF profile hook. Recreate it from trn_agent_boot's
    ctypes shim against the injected libaxon_pjrt.so."""
    import sys as _sys
    import types

    if "antenv.axon_hooks" in _sys.modules:
        return
    try:
        from trn_agent_boot.trn_boot import _ntff_profile_via_ctypes

        hook = _ntff_profile_via_ctypes("/opt/axon/libaxon_pjrt.so")
    except Exception:
        hook = None
    mod = types.ModuleType("antenv.axon_hooks")
    mod._hook = hook
    mod.get_axon_ntff_profile_hook = lambda: mod._hook
    mod.set_axon_ntff_profile_hook = lambda h: setattr(mod, "_hook", h)
    _sys.modules["antenv.axon_hooks"] = mod


def kernel(
    mu_raw, D_over_mu_raw, T0_over_mu_raw, Ly_raw, xo_raw, yo_raw, num_samples
) -> np.ndarray:
    mu_raw = float(np.asarray(mu_raw))
    D_raw = float(np.asarray(D_over_mu_raw))
    T0_raw = float(np.asarray(T0_over_mu_raw))
    Ly_raw = float(np.asarray(Ly_raw))
    xo_raw = float(np.asarray(xo_raw))
    yo_raw = float(np.asarray(yo_raw))
    T = int(np.asarray(num_samples))

    omega, sigma, A = _mode_tables(mu_raw, D_raw, T0_raw, Ly_raw, xo_raw, yo_raw)
    n_valid = omega.shape[0]
    per_core = ((n_valid + N_CORES * 128 - 1) // (N_CORES * 128)) * 128
    n_tiles = per_core // 128
    n_pad = per_core * N_CORES
    omega = np.pad(omega, (0, n_pad - n_valid))
    sigma = np.pad(sigma, (0, n_pad - n_valid))
    A = np.pad(A, (0, n_pad - n_valid))

    nch = (T + C - 1) // C
    pad_di = T - C * (nch - 1)  # valid d's in last chunk; 128 if exact fit

    # host tables in f64, cast to f32
    d = np.arange(C, dtype=np.float64)
    ph = omega[:, None] * K * d[None, :]
    env = np.exp(-sigma[:, None] * K * d[None, :])
    F = (env * np.cos(ph)).astype(np.float32)  # [n_pad, C]
    G = (env * np.sin(ph)).astype(np.float32)

    t0 = np.arange(nch, dtype=np.float64) * C
    th = omega[:, None] * K * t0[None, :]
    cenv = A[:, None] * np.exp(-sigma[:, None] * K * (t0[None, :] - 1.0))
    a = (cenv * np.sin(th)).astype(np.float32)  # [n_pad, nch]
    b = (cenv * np.cos(th)).astype(np.float32)

    nc = _build_nc(n_tiles, nch, pad_di)

    in_maps = []
    for r in range(N_CORES):
        sl = slice(r * per_core, (r + 1) * per_core)
        basis = np.concatenate(
            [_tile_pack(F[sl], n_tiles), _tile_pack(G[sl], n_tiles)], axis=1
        )
        coef = np.concatenate(
            [_tile_pack(a[sl], n_tiles), _tile_pack(b[sl], n_tiles)], axis=1
        )
        in_maps.append(
            {
                "basis": np.ascontiguousarray(basis),
                "coef": np.ascontiguousarray(coef),
            }
        )

    import os

    trace = bool(os.environ.get("MODAL_KERNEL_TRACE"))
    if trace:
        _install_ntff_hook_shim()
    res = run_bass_kernel_spmd(
        nc, in_maps, core_ids=list(range(N_CORES)), trace=trace
    )
    kernel._last_results = res  # for profiling from test.py
    out = res.results[0]["disp"]  # [128, nch], element (d, c) = disp[C*c+d]
    return np.ascontiguousarray(out.T.reshape(-1)[:T]).astype(np.float32)


if __name__ == "__main__":
    z = np.zeros((), np.float32)
    y = kernel(z, z, z, z, z, z, 22050)
    print(y.shape, y.dtype, y[:5], np.max(np.abs(y)))
